# revision 3
# baseline (speedup 1.0000x reference)
"""Bilinear affine image sampling on 8 Trainium2 cores (data parallel over N).

All compute on device; host only moves bytes (this container's host is
single-core at ~0.1 GB/s, so any host pass over the 400MB tensors costs
seconds). Per image n (core k owns images 4k..4k+3):

  1. upload I[n] (3.1MB) + A[n], T[n] to core k                  [async]
  2. XLA prep jit: sample coords hx,hy; x0=clip(floor(hx),0,H-2),
     wx=clip(hx-x0,0,1) (equivalent to the reference's clamp-to-border
     semantics), CHW->HWC transpose                              [device]
  3. XLA gather jit: ONE lax.gather with slice_sizes (2,2,3) fetches all
     four bilinear corners for every output pixel                [device]
  4. Bass/Tile kernel (concourse.bass2jax.bass_jit — the same bass_exec
     custom-call path run_bass_kernel_spmd uses under axon) does the full
     bilinear blend for the core's 4 images on the vector engine [device]

The 8 per-core outputs are viewed as one global sharded jax array and
materialized with a single np.asarray — one host pass, no reassembly.

(Per-element gather inside raw Bass is not viable on this runtime: gpsimd
ap_gather ~162ns/index, Pool INDIRECT_COPY crashes, and the toolchain
disables vector_dynamic_offsets DGE; the gather therefore runs as an XLA
op on the NeuronCores while the blend arithmetic runs in the Bass kernel.)
"""

import sys
import numpy as np

sys.path.insert(0, "/opt/trn_rl_repo")

N, C, H, W = 32, 3, 512, 512
NCORES = 8
IPC = N // NCORES        # images per core
HW = H * W
F = HW // 128            # 2048 free elems per partition

_cache = {}


def _build():
    import jax
    import jax.numpy as jnp
    from jax import lax
    from jax.sharding import Mesh, PartitionSpec, NamedSharding
    from concourse.bass2jax import bass_jit
    import concourse.mybir as mybir
    from concourse import tile

    devs = jax.devices()[:NCORES]
    mesh = Mesh(np.asarray(devs), ("core",))
    gsharding = NamedSharding(mesh, PartitionSpec("core"))

    @jax.jit
    def prep(img, a, t):
        # img [3,512,512] f32, a [2,2], t [2]
        f32 = jnp.float32
        cx = f32((H - 1) / 2.0)
        cy = f32((W - 1) / 2.0)
        xi = (jnp.arange(H, dtype=f32) - cx)[:, None]
        yj = (jnp.arange(W, dtype=f32) - cy)[None, :]
        hx = a[0, 0] * xi + a[0, 1] * yj + t[0] + cx
        hy = a[1, 0] * xi + a[1, 1] * yj + t[1] + cy
        x0 = jnp.clip(jnp.floor(hx), 0.0, f32(H - 2))
        y0 = jnp.clip(jnp.floor(hy), 0.0, f32(W - 2))
        wx = jnp.clip(hx - x0, 0.0, 1.0)
        wy = jnp.clip(hy - y0, 0.0, 1.0)
        starts = jnp.stack(
            [x0.astype(jnp.int32).reshape(HW), y0.astype(jnp.int32).reshape(HW)],
            axis=1)
        hwc = jnp.transpose(img, (1, 2, 0))
        return hwc, starts, wx.reshape(128, F), wy.reshape(128, F)

    dn = lax.GatherDimensionNumbers(
        offset_dims=(1, 2, 3), collapsed_slice_dims=(), start_index_map=(0, 1))

    @jax.jit
    def corners(hwc, starts):
        g = lax.gather(hwc, starts, dn, slice_sizes=(2, 2, C),
                       mode=lax.GatherScatterMode.PROMISE_IN_BOUNDS)
        return g.reshape(128, F, 2 * 2 * C)

    @bass_jit(trn_type="TRN2")
    def blend(nc, g0, g1, g2, g3, wx0, wx1, wx2, wx3, wy0, wy1, wy2, wy3):
        # g* [128,F,12] f32 (pixel-major; last dim = corner(2x2) x channel),
        # wx*/wy* [128,F] f32 -> out [IPC,C,128,F]
        gs = [g0, g1, g2, g3]
        wxs = [wx0, wx1, wx2, wx3]
        wys = [wy0, wy1, wy2, wy3]
        out_d = nc.dram_tensor(
            "out", [IPC, C, 128, F], mybir.dt.float32, kind="ExternalOutput")
        with tile.TileContext(nc) as tc:
            with tc.tile_pool(name="sbuf", bufs=1) as pool:
                for m in range(IPC):
                    gt = pool.tile([128, F, 12], mybir.dt.float32, tag="g")
                    wxt = pool.tile([128, F], mybir.dt.float32, tag="wx")
                    wyt = pool.tile([128, F], mybir.dt.float32, tag="wy")
                    t01 = pool.tile([128, F], mybir.dt.float32, tag="t01")
                    t11 = pool.tile([128, F], mybir.dt.float32, tag="t11")
                    nc.sync.dma_start(gt[:, :, :], gs[m][:, :, :])
                    nc.sync.dma_start(wxt[:], wxs[m][:, :])
                    nc.sync.dma_start(wyt[:], wys[m][:, :])
                    for c in range(C):
                        oc = pool.tile([128, F], mybir.dt.float32, tag=f"o{c}")
                        p00 = gt[:, :, 0 * C + c]
                        p01 = gt[:, :, 1 * C + c]
                        p10 = gt[:, :, 2 * C + c]
                        p11 = gt[:, :, 3 * C + c]
                        # top = p00 + wy*(p01-p00)
                        nc.vector.tensor_tensor(t01[:], p01, p00, mybir.AluOpType.subtract)
                        nc.vector.tensor_tensor(t01[:], t01[:], wyt[:], mybir.AluOpType.mult)
                        nc.vector.tensor_tensor(t01[:], t01[:], p00, mybir.AluOpType.add)
                        # bot = p10 + wy*(p11-p10)
                        nc.vector.tensor_tensor(t11[:], p11, p10, mybir.AluOpType.subtract)
                        nc.vector.tensor_tensor(t11[:], t11[:], wyt[:], mybir.AluOpType.mult)
                        nc.vector.tensor_tensor(t11[:], t11[:], p10, mybir.AluOpType.add)
                        # out = top + wx*(bot-top)
                        nc.vector.tensor_tensor(t11[:], t11[:], t01[:], mybir.AluOpType.subtract)
                        nc.vector.tensor_tensor(t11[:], t11[:], wxt[:], mybir.AluOpType.mult)
                        nc.vector.tensor_tensor(oc[:], t11[:], t01[:], mybir.AluOpType.add)
                        nc.sync.dma_start(out_d[m, c], oc[:])
        return out_d

    @jax.jit
    def to_final(o):
        # [IPC,C,128,F] -> [IPC,C,512,512] (same bytes)
        return o.reshape(IPC, C, H, W)

    _cache.update(
        jax=jax, devs=devs, gsharding=gsharding,
        prep=prep, corners=corners, blend=blend, to_final=to_final,
    )


def kernel(I, A, T):
    if not _cache:
        _build()
    jax = _cache["jax"]
    devs = _cache["devs"]
    prep = _cache["prep"]
    corners = _cache["corners"]
    blend = _cache["blend"]
    to_final = _cache["to_final"]

    I = np.asarray(I)
    if I.dtype != np.float32:
        I = I.astype(np.float32)
    A = np.asarray(A, dtype=np.float32)
    T = np.asarray(T, dtype=np.float32)

    # dispatch everything asynchronously, image-major so all cores start early
    gq = [[None] * IPC for _ in range(NCORES)]
    wxs = [[None] * IPC for _ in range(NCORES)]
    wys = [[None] * IPC for _ in range(NCORES)]
    for m in range(IPC):
        for k in range(NCORES):
            n = k * IPC + m
            img = jax.device_put(I[n], devs[k])
            a = jax.device_put(A[n], devs[k])
            t = jax.device_put(T[n], devs[k])
            hwc, starts, wx, wy = prep(img, a, t)
            gq[k][m] = corners(hwc, starts)
            wxs[k][m] = wx
            wys[k][m] = wy

    packed = []
    for k in range(NCORES):
        o = blend(*gq[k], *wxs[k], *wys[k])
        packed.append(to_final(o))

    garr = jax.make_array_from_single_device_arrays(
        (N, C, H, W), _cache["gsharding"], packed)
    return np.asarray(garr)
